# revision 19
# baseline (speedup 1.0000x reference)
"""Trainium2 Bass kernel for multi-head attention (GQA + RoPE + causal).

Problem shapes (hardcoded): x [2, 2048, 2048] f32, w_qkv [3072, 2048],
w_o [2048, 2048], position_ids [2, 2048] int, mask [1,1,2048,2048] causal.

Sharding: 8 cores = 2 batches x 4 KV-head groups. Each core computes, for
one batch b and one kv-group g (4 query heads + 1 kv head):
  - Y^T = (w_shard @ x[b]^T) in "feature-major" layout [f, s]
  - RoPE on Q^T/K^T (tables precomputed on host from position_ids)
  - causal attention in transposed-score layout S_T[k, q] (no transposes,
    softmax sums via ones-vector matmul; no max subtraction needed at these
    score magnitudes |s| < ~10)
  - partial o_proj out^T[oc, s] = w_o_slice^T @ A^T  (bf16 partial)
Host sums the 4 partials per batch and transposes back.

Projections (qkv and o_proj) run as error-compensated fp8 DoubleRow
matmuls: operands are split hi/lo into e4m3 (x, w_qkv, w_o on host; the
attention output A on-device via DVE), and each 256-deep contraction pair
computes hi*hi + hi*lo + lo*hi at 0.5 cycles/row, i.e. 0.75x the PE time
of bf16 at slightly BETTER-than-bf16 accuracy (lo*lo term ~0.13% is
dropped). The attention core (scores, exp-sums, PV) stays bf16: a single
e4m3 quantization of p/q/k/v adds ~3-5% output error, over tolerance.
"""

import math
from contextlib import ExitStack
from dataclasses import dataclass

import numpy as np
import ml_dtypes

import concourse.bass as bass
import concourse.tile as tile
from concourse import bacc, mybir
from concourse.masks import make_identity

P = 128
BF16 = mybir.dt.bfloat16
F32 = mybir.dt.float32
F8 = mybir.dt.float8e4
BF16_NP = ml_dtypes.bfloat16
F8_NP = ml_dtypes.float8_e4m3
DR = mybir.MatmulPerfMode.DoubleRow

# full-size problem constants
B, S_FULL, HID_FULL = 2, 2048, 2048
NH, NKV, HD = 16, 4, 128
NQL_HD = (NH // NKV) * HD  # 512
ROPE_BASE = 10000.0
N_CORES = 8
WSCALE = 2048.0   # fp8 pre-scale for w_qkv (values ~0.02*randn)
WOSCALE = 2048.0  # fp8 pre-scale for w_o


@dataclass(frozen=True)
class Cfg:
    S: int = S_FULL          # sequence length
    HID: int = HID_FULL      # model dim (contraction for qkv proj)
    NQL: int = NH // NKV     # local query heads per core
    QT: int = 512            # q tile (matmul free dim)
    KG: int = 1              # k-tiles per exp group
    phases: str = "pako"     # debug: p=proj, a=attention, k=skip?, o=oproj

    @property
    def HT(self):            # contraction tiles for qkv proj
        return self.HID // P

    @property
    def NQT(self):           # q tiles per head
        return self.S // self.QT

    @property
    def NKT(self):           # k tiles (128 wide)
        return self.S // P

    @property
    def FQKV(self):          # 128-blocks of qkv features (4 q + 1 k + 1 v)
        return self.NQL + 2

    @property
    def OC(self):            # o_proj output features (full hidden)
        return self.HID

    @property
    def TPQ(self):           # k tiles per q tile (causal step)
        return self.QT // P


def emit(ctx: ExitStack, tc: tile.TileContext, cfg: Cfg, io: dict, n_reps: int = 1):
    res = ctx.enter_context(tc.tile_pool(name="res", bufs=1))
    work = ctx.enter_context(tc.tile_pool(name="work", bufs=1))
    ps = ctx.enter_context(tc.tile_pool(name="ps", bufs=1, space="PSUM"))
    for rep in range(n_reps):  # >1 only for timing builds
        # accumulate into outT on reps > 0 so repeats aren't dead-code
        # eliminated by the NEFF compiler (timing builds only)
        emit_once(tc, cfg, io, res, work, ps, accum=(rep > 0))


def emit_once(tc: tile.TileContext, cfg: Cfg, io: dict, res, work, ps, accum=False):
    nc = tc.nc
    S, QT, KG, HT, NQL = cfg.S, cfg.QT, cfg.KG, cfg.HT, cfg.NQL
    NS = S // QT  # s slices of size QT for projection loops
    xTh, xTl, wqkh, wqkl, woh, wol, cosT, sinT, ones_col, outT = (
        io["xTh"], io["xTl"], io["wqkh"], io["wqkl"], io["woh"], io["wol"],
        io["cosT"], io["sinT"], io["ones_col"], io["outT"],
    )
    tri = io["tri"]

    # chunked loads (per h-tile pair) so the first matmuls can start early
    xh_sb = res.tile([P, HT, S], F8, tag="xh")
    xl_sb = res.tile([P, HT, S], F8, tag="xl")
    xh_r = xTh.rearrange("(ht p) s -> p ht s", p=P)
    xl_r = xTl.rearrange("(ht p) s -> p ht s", p=P)
    wqkh_sb = res.tile([P, HT, cfg.FQKV * P], F8, tag="wqkh")
    wqkl_sb = res.tile([P, HT, cfg.FQKV * P], F8, tag="wqkl")
    wqkh_r = wqkh.rearrange("(ht p) f -> p ht f", p=P)
    wqkl_r = wqkl.rearrange("(ht p) f -> p ht f", p=P)
    # DMA order: the K-projection is the critical path (attention needs all
    # of K before any q head finishes), so per chunk send K,V weight slices
    # and the x chunk; q weight columns follow once x is fully issued.
    CH = max(1, HT // 8)  # 8 chunks
    kvf = slice(NQL * P, cfg.FQKV * P)  # K,V feature columns
    for h0 in range(0, HT, CH):
        hs = slice(h0, h0 + CH)
        nc.sync.dma_start(out=wqkh_sb[:, hs, kvf], in_=wqkh_r[:, hs, kvf])
        nc.sync.dma_start(out=wqkl_sb[:, hs, kvf], in_=wqkl_r[:, hs, kvf])
        nc.sync.dma_start(out=xh_sb[:, hs, :], in_=xh_r[:, hs, :])
        nc.sync.dma_start(out=xl_sb[:, hs, :], in_=xl_r[:, hs, :])
    qf = slice(0, NQL * P)  # Q feature columns
    for h0 in range(0, HT, CH):
        hs = slice(h0, h0 + CH)
        nc.sync.dma_start(out=wqkh_sb[:, hs, qf], in_=wqkh_r[:, hs, qf])
        nc.sync.dma_start(out=wqkl_sb[:, hs, qf], in_=wqkl_r[:, hs, qf])
    cos_sb = res.tile([P, S], BF16, tag="cos")
    sin_sb = res.tile([P, S], BF16, tag="sin")
    nc.sync.dma_start(out=cos_sb[:], in_=cosT[:, :])
    nc.sync.dma_start(out=sin_sb[:], in_=sinT[:, :])
    ones_c_sb = res.tile([P, 1], BF16, tag="onesc")
    nc.sync.dma_start(out=ones_c_sb[:], in_=ones_col[:, :])
    tri_sb = res.tile([P, P], BF16, tag="tri")
    nc.sync.dma_start(out=tri_sb[:], in_=tri[:, :])
    ident_sb = res.tile([P, P], BF16, tag="ident")
    make_identity(nc, ident_sb[:])
    woh_sb = res.tile([P, NQL, cfg.OC], F8, tag="woh")
    wol_sb = res.tile([P, NQL, cfg.OC], F8, tag="wol")
    nc.sync.dma_start(out=woh_sb[:], in_=woh.rearrange("(fq p) oc -> p fq oc", p=P))
    nc.sync.dma_start(out=wol_sb[:], in_=wol.rearrange("(fq p) oc -> p fq oc", p=P))

    qT_sb = res.tile([P, NQL, S], BF16, tag="qT")   # roped, pre-scaled Q^T
    kT_sb = res.tile([P, S], BF16, tag="kT")        # roped K^T
    v_sb = res.tile([P, cfg.NKT, P], BF16, tag="v")  # V natural [s-part, v]
    ah_sb = res.tile([P, NQL, S], F8, tag="ah")     # attention out A^T hi
    al_sb = res.tile([P, NQL, S], F8, tag="al")     # attention out A^T lo

    # ---- projection helper (Y^T for one 128-wide feature block) ----
    # compensated fp8 DoubleRow: per 256-contraction pair
    #   acc += wh.T@xh + wl.T@xh + wh.T@xl   (the wl.T@xl term is dropped)
    def proj_block(fslice, si, dst, do_rope):
        sl = bass.ts(si, QT)
        acc = ps.tile([P, QT], F32, tag="mm", bufs=2, name="acc")
        NP2 = HT // 2
        for hp in range(NP2):
            pr = slice(2 * hp, 2 * hp + 2)
            nc.tensor.matmul(acc[:], wqkh_sb[:, pr, fslice], xh_sb[:, pr, sl],
                             start=(hp == 0), stop=False, perf_mode=DR)
            nc.tensor.matmul(acc[:], wqkl_sb[:, pr, fslice], xh_sb[:, pr, sl],
                             start=False, stop=False, perf_mode=DR)
            nc.tensor.matmul(acc[:], wqkh_sb[:, pr, fslice], xl_sb[:, pr, sl],
                             start=False, stop=(hp == NP2 - 1), perf_mode=DR)
        y = work.tile([P, QT], BF16, tag="y", bufs=6, name="y")
        nc.scalar.mul(y[:], acc[:], 1.0 / WSCALE)
        if not do_rope:
            return y
        # rope: out = y*cos + swap_halves(y)*sin'
        # (sin' is pre-negated in its lower half on host).
        # Half-swap via 1-input copies: 2-input DVE ops require equal
        # SBUF base partitions on HW.
        sw = work.tile([P, QT], BF16, tag="sw", bufs=4, name="sw")
        nc.vector.tensor_copy(sw[0:64, :], y[64:128, :])
        nc.vector.tensor_copy(sw[64:128, :], y[0:64, :])
        t1 = work.tile([P, QT], BF16, tag="t1", bufs=4, name="t1")
        nc.vector.tensor_mul(t1[:], sw[:], sin_sb[:, sl])
        t2 = work.tile([P, QT], BF16, tag="t2", bufs=4, name="t2")
        nc.vector.tensor_mul(t2[:], y[:], cos_sb[:, sl])
        nc.vector.tensor_add(dst, t2[:], t1[:])
        return None

    # ---- K^T projection first (attention needs it before q heads) ----
    with nc.named_scope("k_proj"):
        for si in range(NS):
            proj_block(bass.ts(NQL, P), si, kT_sb[:, bass.ts(si, QT)], True)

    # ---- V^T projection + transpose to natural V ----
    with nc.named_scope("v_proj"):
        for si in range(NS):
            vt = proj_block(bass.ts(NQL + 1, P), si, None, False)
            for j in range(QT // P):
                pst = ps.tile([P, P], BF16, tag="mm", bufs=2, name="pst")
                nc.tensor.transpose(pst[:], vt[:, bass.ts(j, P)], ident_sb[:])
                nc.scalar.copy(v_sb[:, si * (QT // P) + j, :], pst[:])

    # ---- Q^T projections ----
    with nc.named_scope("q_proj"):
        for fi in range(NQL):
            for si in range(NS):
                proj_block(bass.ts(fi, P), si,
                           qT_sb[:, fi, bass.ts(si, QT)], True)

    # ---- attention + o_proj interleaved per q tile ----
    def o_proj_tile(t):
        qsl = bass.ts(t, QT)
        for oi in range(cfg.OC // P if "o" in cfg.phases else 0):
            osl = bass.ts(oi, P)
            acc = ps.tile([P, QT], F32, tag="mm", bufs=2, name="acc_o")
            NG2 = NQL // 2
            for gp in range(NG2):
                pr = slice(2 * gp, 2 * gp + 2)
                nc.tensor.matmul(acc[:], woh_sb[:, pr, osl], ah_sb[:, pr, qsl],
                                 start=(gp == 0), stop=False, perf_mode=DR)
                nc.tensor.matmul(acc[:], wol_sb[:, pr, osl], ah_sb[:, pr, qsl],
                                 start=False, stop=False, perf_mode=DR)
                nc.tensor.matmul(acc[:], woh_sb[:, pr, osl], al_sb[:, pr, qsl],
                                 start=False, stop=(gp == NG2 - 1), perf_mode=DR)
            orow = work.tile([P, QT], BF16, tag="orow", bufs=4, name="orow")
            if accum and oi == 0 and t == 0:
                # timing builds: chain on previous rep's output so the
                # NEFF compiler can't dead-code-eliminate earlier reps
                prev = work.tile([P, QT], BF16, tag="prev", bufs=1, name="prev")
                nc.sync.dma_start(out=prev[:], in_=outT[0:P, 0:QT])
                o32 = work.tile([P, QT], F32, tag="o32", bufs=1, name="o32")
                nc.vector.tensor_scalar_mul(o32[:], acc[:], 1.0 / WOSCALE)
                nc.vector.tensor_add(orow[:], o32[:], prev[:])
            elif oi % 2 == 0:
                # alternate the PSUM drain between DVE and Act so neither
                # engine's copy stream gates PSUM buffer recycling
                nc.vector.tensor_scalar_mul(orow[:], acc[:], 1.0 / WOSCALE)
            else:
                nc.scalar.mul(orow[:], acc[:], 1.0 / WOSCALE)
            nc.sync.dma_start(out=outT[osl, qsl], in_=orow[:])

    with nc.named_scope("attn"):
        for t in range(cfg.NQT):
            qsl = bass.ts(t, QT)
            for h in range(NQL if "a" in cfg.phases else 0):
                nk = (t + 1) * cfg.TPQ  # valid k tiles (causal)
                groups = [list(range(g, min(g + KG, nk))) for g in range(0, nk, KG)]
                pv_ps = ps.tile([P, QT], F32, tag="pv", bufs=2, name="pv_ps")
                sums_ps = ps.tile([1, QT], F32, tag="sums", bufs=1, name="sums_ps")
                # off-diagonal k tiles (fully unmasked) run in pairs: one
                # merged exp + one merged sums matmul per pair. Diagonal-band
                # tiles (delta > 0 possible) run as singletons.
                groups = [[j] for j in range(nk)]
                # software pipeline: emit scores+exp for group idx, and
                # sums/pv for group idx-LA, so the PE never sits at the head
                # of the queue waiting for an exp that was just issued.
                LA = 4
                first = True
                pend = []  # (p_sb tile, ks, deltas)

                def consume(entry):
                    nonlocal first
                    pp, ks_, deltas_ = entry
                    for i, j in enumerate(ks_):
                        d = deltas_[i]
                        last = j == nk - 1
                        nc.tensor.matmul(
                            sums_ps[:, d:QT], ones_c_sb[:, :], pp[:, i, d:QT],
                            start=first, stop=last,
                        )
                        nc.tensor.matmul(
                            pv_ps[:, d:QT], v_sb[:, j, :], pp[:, i, d:QT],
                            start=first, stop=last,
                        )
                        first = False

                for gi, ks in enumerate(groups):
                    s_ps = ps.tile([P, 1, QT], F32, tag="s", bufs=3, name="s_ps")
                    p_sb = work.tile([P, 1, QT], BF16, tag="p", bufs=8, name="p_sb")
                    # boundary tiles (delta >= 0): columns q' < delta are fully
                    # masked, so restrict the whole chain to [delta:QT].
                    deltas = [max(0, j * P - t * QT) for j in ks]
                    for i, j in enumerate(ks):
                        d = deltas[i]
                        nc.tensor.matmul(
                            s_ps[:, i, d:QT],
                            kT_sb[:, bass.ts(j, P)],
                            qT_sb[:, h, t * QT + d:(t + 1) * QT],
                            start=True, stop=True,
                        )
                    if len(ks) == 2:
                        nc.scalar.activation(
                            p_sb[:, 0:2, :], s_ps[:, 0:2, :],
                            mybir.ActivationFunctionType.Exp,
                        )
                    else:
                        d = deltas[0]
                        nc.scalar.activation(
                            p_sb[:, 0, d:QT], s_ps[:, 0, d:QT],
                            mybir.ActivationFunctionType.Exp,
                        )
                        if ks[0] * P - t * QT >= 0:
                            # diagonal 128-block keeps q' >= k' + delta, i.e.
                            # the base (delta=0) triangle at offset delta
                            nc.vector.tensor_mul(
                                p_sb[:, 0, d:d + P], p_sb[:, 0, d:d + P],
                                tri_sb[:, :],
                            )
                    pend.append((p_sb, ks, deltas))
                    if len(pend) > LA:
                        consume(pend.pop(0))
                for entry in pend:
                    consume(entry)
                recip = work.tile([1, QT], F32, tag="recip", bufs=3, name="recip")
                nc.vector.reciprocal(recip[:], sums_ps[:])
                bc_sb = work.tile([P, QT], F32, tag="bc", bufs=3, name="bc_sb")
                nc.gpsimd.partition_broadcast(bc_sb[:], recip[:], channels=P)
                a32 = work.tile([P, QT], F32, tag="a32", bufs=3, name="a32")
                nc.vector.tensor_mul(a32[:], pv_ps[:], bc_sb[:])
                nc.vector.tensor_copy(ah_sb[:, h, qsl], a32[:])
                nc.vector.tensor_sub(al_sb[:, h, qsl], a32[:], ah_sb[:, h, qsl])
            # o_proj pipelined one q-tile behind attention: emit o_proj(t-1)
            # here so tile t's scores hide the head-3 normalize latency
            if t > 0:
                o_proj_tile(t - 1)
        o_proj_tile(cfg.NQT - 1)


def build(cfg: Cfg, n_reps: int = 1):
    nc = bacc.Bacc("TRN2", target_bir_lowering=False, debug=False)
    io = {
        "xTh": nc.dram_tensor("xTh", [cfg.HID, cfg.S], F8, kind="ExternalInput").ap(),
        "xTl": nc.dram_tensor("xTl", [cfg.HID, cfg.S], F8, kind="ExternalInput").ap(),
        "wqkh": nc.dram_tensor("wqkh", [cfg.HID, cfg.FQKV * P], F8, kind="ExternalInput").ap(),
        "wqkl": nc.dram_tensor("wqkl", [cfg.HID, cfg.FQKV * P], F8, kind="ExternalInput").ap(),
        "woh": nc.dram_tensor("woh", [cfg.NQL * P, cfg.OC], F8, kind="ExternalInput").ap(),
        "wol": nc.dram_tensor("wol", [cfg.NQL * P, cfg.OC], F8, kind="ExternalInput").ap(),
        "cosT": nc.dram_tensor("cosT", [P, cfg.S], BF16, kind="ExternalInput").ap(),
        "sinT": nc.dram_tensor("sinT", [P, cfg.S], BF16, kind="ExternalInput").ap(),
        "ones_col": nc.dram_tensor("ones_col", [P, 1], BF16, kind="ExternalInput").ap(),
        "tri": nc.dram_tensor("tri", [P, P], BF16, kind="ExternalInput").ap(),
        "outT": nc.dram_tensor("outT", [cfg.OC, cfg.S], BF16, kind="ExternalOutput").ap(),
    }
    with tile.TileContext(nc) as tc:
        with ExitStack() as ctx:
            emit(ctx, tc, cfg, io, n_reps=n_reps)
    nc.compile()
    return nc


def rope_tables(position_ids_b: np.ndarray):
    """cos/sin tables in [d, s] layout, both halves stacked; sin lower half
    negated (so rope = y*cos + swap(y)*sin)."""
    half = HD // 2
    inv_freq = 1.0 / (ROPE_BASE ** (np.arange(half, dtype=np.float64) / half))
    freqs = np.asarray(position_ids_b, dtype=np.float64)[None, :] * inv_freq[:, None]
    cos = np.cos(freqs)
    sin = np.sin(freqs)
    cosT = np.concatenate([cos, cos], 0)
    sinT = np.concatenate([-sin, sin], 0)
    return cosT, sinT


def _split_fp8(a: np.ndarray):
    """hi/lo e4m3 split: a ~= hi + lo (both e4m3)."""
    hi = a.astype(F8_NP)
    lo = (a - hi.astype(np.float32)).astype(F8_NP)
    return hi, lo


def make_in_maps(x, position_ids, w_qkv, w_o):
    """Shard full inputs into per-core input maps (host-side prep)."""
    q_dim = NH * HD
    kv_dim = NKV * HD
    in_maps = []
    ones_col = np.ones((P, 1), dtype=BF16_NP)
    tri = make_tri()
    scale = 1.0 / math.sqrt(HD)
    tabs = {}
    for b in range(B):
        cosT, sinT = rope_tables(position_ids[b])
        tabs[b] = (cosT.astype(BF16_NP), sinT.astype(BF16_NP))
    for c in range(N_CORES):
        b, g = divmod(c, NKV)
        # weights for this core's heads: 4 q heads (pre-scaled), 1 k, 1 v head
        wq = w_qkv[g * NQL_HD:(g + 1) * NQL_HD, :] * scale
        wk = w_qkv[q_dim + g * HD:q_dim + (g + 1) * HD, :]
        wv = w_qkv[q_dim + kv_dim + g * HD:q_dim + kv_dim + (g + 1) * HD, :]
        wqkT = np.ascontiguousarray(
            np.concatenate([wq, wk, wv], 0).T).astype(np.float32) * WSCALE
        wqkh, wqkl = _split_fp8(wqkT)
        # o_proj: rows of w_o^T for this core's flattened head features
        woT = np.ascontiguousarray(
            w_o.T[g * NQL_HD:(g + 1) * NQL_HD, :]).astype(np.float32) * WOSCALE
        woh, wol = _split_fp8(woT)
        xT = np.ascontiguousarray(x[b].T).astype(np.float32)
        xTh, xTl = _split_fp8(xT)
        in_maps.append({
            "xTh": xTh,
            "xTl": xTl,
            "wqkh": wqkh,
            "wqkl": wqkl,
            "woh": woh,
            "wol": wol,
            "cosT": tabs[b][0],
            "sinT": tabs[b][1],
            "ones_col": ones_col,
            "tri": tri,
        })
    return in_maps


def make_tri():
    """Diagonal-block causal mask: tri[k, q] = 1 if q >= k."""
    k = np.arange(P)
    q = np.arange(P)
    return (q[None, :] >= k[:, None]).astype(BF16_NP)


def _causal_mask_ok(mask):
    m = np.asarray(mask)
    if m.shape != (1, 1, S_FULL, S_FULL):
        return False
    tril = np.tril(np.ones((S_FULL, S_FULL), dtype=bool))
    m0 = m[0, 0]
    return bool((m0[tril] == 0.0).all() and (m0[~tril] <= -1e8).all())


def _reference_numpy(x, position_ids, mask, w_qkv, w_o):
    """Fallback (never expected to trigger): plain numpy reference."""
    half = HD // 2

    def rope(v, pos):
        inv_freq = 1.0 / (ROPE_BASE ** (np.arange(half) / half))
        f = np.asarray(pos, dtype=np.float64)[:, None] * inv_freq[None, :]
        cos, sin = np.cos(f), np.sin(f)
        x1, x2 = v[..., :half], v[..., half:]
        return np.concatenate([x1 * cos - x2 * sin, x2 * cos + x1 * sin], -1)

    out = np.empty((B, S_FULL, HID_FULL), np.float32)
    q_dim, kv_dim = NH * HD, NKV * HD
    xd = x.astype(np.float64)
    for b in range(B):
        qkv = xd[b] @ w_qkv.T.astype(np.float64)
        q = qkv[:, :q_dim].reshape(S_FULL, NH, HD).transpose(1, 0, 2)
        k = qkv[:, q_dim:q_dim + kv_dim].reshape(S_FULL, NKV, HD).transpose(1, 0, 2)
        v = qkv[:, q_dim + kv_dim:].reshape(S_FULL, NKV, HD).transpose(1, 0, 2)
        q = np.stack([rope(qh, position_ids[b]) for qh in q])
        k = np.stack([rope(kh, position_ids[b]) for kh in k])
        rep = NH // NKV
        acc = np.empty((S_FULL, NH, HD))
        for h in range(NH):
            s = q[h] @ k[h // rep].T / math.sqrt(HD) + mask[0, 0]
            s -= s.max(-1, keepdims=True)
            e = np.exp(s)
            p = e / e.sum(-1, keepdims=True)
            acc[:, h, :] = p @ v[h // rep]
        out[b] = (acc.reshape(S_FULL, NH * HD) @ w_o.T.astype(np.float64)).astype(np.float32)
    return out


_NC_CACHE = {}


def _get_nc():
    if "full" not in _NC_CACHE:
        _NC_CACHE["full"] = build(Cfg())
    return _NC_CACHE["full"]


def kernel(x, position_ids, mask, w_qkv, w_o):
    x = np.asarray(x, dtype=np.float32)
    position_ids = np.asarray(position_ids)
    w_qkv = np.asarray(w_qkv, dtype=np.float32)
    w_o = np.asarray(w_o, dtype=np.float32)
    if not _causal_mask_ok(mask):
        return _reference_numpy(x, position_ids, np.asarray(mask, np.float32),
                                w_qkv, w_o)

    from concourse.bass_utils import run_bass_kernel_spmd

    nc = _get_nc()
    in_maps = make_in_maps(x, position_ids, w_qkv, w_o)
    res = run_bass_kernel_spmd(nc, in_maps, list(range(N_CORES)))
    out = np.empty((B, S_FULL, HID_FULL), dtype=np.float32)
    for b in range(B):
        acc = res.results[b * NKV + 0]["outT"].astype(np.float32)
        for g in range(1, NKV):
            acc = acc + res.results[b * NKV + g]["outT"].astype(np.float32)
        out[b] = acc.T
    return out
